# revision 19
# baseline (speedup 1.0000x reference)
# Trainium2 Bass kernel for nn_CustomStyleLoss (segment-mean + MSE reduction).
#
# loss = sum_rows mean_chunks( (mean_chunk(input) - mean_chunk(style))^2 )
# with rows = 16*512 = 8192, each row = 50*50 = 2500 elems = 25 chunks of 100.
#
# Data-parallel over rows: core i gets rows [i*1024, (i+1)*1024).
#
# Strategy (memory regime; measured on HW at each step):
# - The 2e-2 tolerance admits bf16 (measured end-to-end rel err 3.6e-5),
#   so the host casts both tensors to bf16, halving HBM traffic to
#   10.24MB/core, and pre-arranges each core shard as [128 partitions,
#   8*2500] (row r=t*128+p -> partition p, column t*2500+k) so every DMA
#   slice is per-partition contiguous at any width.
# - x pieces stream on the SP HWDGE ring, style pieces on the ACT ring
#   (~425-435 GB/s aggregate measured, i.e. ~1.18ns per column of both
#   tensors). Everything fits in SBUF at once (95KB/partition), so DMA
#   free-runs with no slot-reuse backpressure. Piece widths ramp small->
#   large->small: the first compute starts at ~10us instead of ~18.5,
#   and the lands pace the DVE with only ~3us of early idle (each piece
#   boundary also costs ~0.5-1us of completion-receipt lag, so more/
#   smaller pieces are not free - 9 pieces measured best).
# - Compute per piece on the DVE at measured-best perf modes: tensor_sub
#   bf16 (2x_1P, 0.52ns/col), the aligned fold d[...,0:50]+d[...,50:100]
#   (2x even on strided 3D views, 0.26ns/col), tensor_reduce [*,50] (1x
#   uop cap, 0.52ns/elem) with fp32 out - ~1.30ns/col total, formula-
#   exact ((N+151)cyc @0.96GHz per op).
# - Rejected on measurement: fp32/bf16 tensor_tensor_scan (2.11ns/elem
#   regardless of dtype); SWDGE accum_op subtract-in-DMA (CCE RMW runs
#   ~193 GB/s solo, ~40 GB/s while HWDGE is busy, and costs 2x port
#   budget); GpSimd tensor_sub offload (its TT runs 1.92ns/elem but
#   concurrent GpSimd compute slows DVE ops 3.4-6.5x - SBUF access
#   interference - a large net loss).
# - Squares+row-sums run on the ACT engine (Square with accum_out) per
#   piece except the final 4-chunk sliver (DVE, skips one cross-engine
#   hop on the tail). Final scale is applied on the host.

import sys

if "/opt/trn_rl_repo" not in sys.path:
    sys.path.insert(0, "/opt/trn_rl_repo")

import ml_dtypes
import numpy as np

import concourse.bass as bass
from concourse import mybir
from concourse.bass_utils import run_bass_kernel_spmd

N_CORES = 8
N_ROWS = 8192
K = 2500
CHUNK = 100
P = 128
ROWS_PER_CORE = N_ROWS // N_CORES    # 1024
N_TILES = ROWS_PER_CORE // P         # 8
W = N_TILES * K                      # 20000 columns in device layout
N_CHUNKS = W // CHUNK                # 200 chunk sums per partition
SCALE2 = 1.0 / (CHUNK * CHUNK * (K // CHUNK))

# Piece widths (columns, chunk-aligned) and owner of the subtract:
# 'V' = DVE does sub+fold+reduce; 'G' = GpSimd does sub, DVE fold+reduce.
# NOTE: 'G' is now unused - measured GpSimd TT runs concurrently with DVE
# ops slow the DVE 3.4-6.5x (SBUF access interference), a net loss.
# Ramp chosen so each piece's full landing precedes the DVE becoming
# ready for it (no mid-kernel DVE idle), given DVE ~1.30ns/col + 0.77us
# fixed per piece vs DMA ~1.18ns/col.
PIECE_W = [400, 1200, 2400, 3200, 3600, 3600, 3600, 1600, 400]
OWNER =   ["V", "V",  "V",  "V",  "V",  "V",  "V",  "V",  "V"]
assert sum(PIECE_W) == W and all(w % CHUNK == 0 for w in PIECE_W)
N_PIECES = len(PIECE_W)
PIECES = []
_c = 0
for _w in PIECE_W:
    PIECES.append((_c, _c + _w))
    _c += _w
G_IDX = [j for j, o in enumerate(OWNER) if o == "G"]
MAX_GW = max((PIECE_W[j] for j in G_IDX), default=CHUNK)

_CACHED_NC = None


def _build_nc():
    nc = bass.Bass(
        "TRN2", target_bir_lowering=False, debug=False, num_devices=N_CORES
    )
    x = nc.dram_tensor(
        "input", [P, W], mybir.dt.bfloat16, kind="ExternalInput"
    ).ap()
    s = nc.dram_tensor(
        "style", [P, W], mybir.dt.bfloat16, kind="ExternalInput"
    ).ap()
    o = nc.dram_tensor(
        "out", [P, N_PIECES], mybir.dt.float32, kind="ExternalOutput"
    ).ap()

    from contextlib import ExitStack

    with ExitStack() as ctx:
        xt = ctx.enter_context(nc.sbuf_tensor("xt", [P, W], mybir.dt.bfloat16))
        st = ctx.enter_context(nc.sbuf_tensor("st", [P, W], mybir.dt.bfloat16))
        dt_ = ctx.enter_context(
            nc.sbuf_tensor("dt", [P, max(PIECE_W)], mybir.dt.bfloat16)
        )
        ft = ctx.enter_context(
            nc.sbuf_tensor("ft", [P, max(PIECE_W) // 2], mybir.dt.bfloat16)
        )
        # GpSimd-owned pieces get their own sub buffer, slotted per piece
        # so the DVE never races a reuse.
        dt_g = ctx.enter_context(
            nc.sbuf_tensor(
                "dt_g", [P, max(len(G_IDX), 1), MAX_GW], mybir.dt.bfloat16
            )
        )
        cs = ctx.enter_context(
            nc.sbuf_tensor("cs", [P, N_CHUNKS], mybir.dt.float32)
        )
        max_nch = max(PIECE_W) // CHUNK
        sqv = ctx.enter_context(
            nc.sbuf_tensor("sqv", [P, max_nch], mybir.dt.float32)
        )
        sq = ctx.enter_context(
            nc.sbuf_tensor("sq", [P, max_nch], mybir.dt.float32)
        )
        partials = ctx.enter_context(
            nc.sbuf_tensor("partials", [P, N_PIECES], mybir.dt.float32)
        )
        s_x = ctx.enter_context(nc.semaphore("s_x"))
        s_sv = ctx.enter_context(nc.semaphore("s_sv"))
        s_g = ctx.enter_context(nc.semaphore("s_g"))
        s_d = ctx.enter_context(nc.semaphore("s_d"))
        s_cs = ctx.enter_context(nc.semaphore("s_cs"))
        s_out = ctx.enter_context(nc.semaphore("s_out"))
        block = ctx.enter_context(nc.Block(no_gpsimd_drain=True))

        def seg(ap2d, k):  # [P, n*k] -> [P, n, k]
            return ap2d.rearrange("p (c k) -> p c k", k=k)

# The ACT HWDGE ring's first packet consistently starts ~2-3us after
        # the SP ring's, so the first EARLY_SP pieces load BOTH tensors on
        # the SP ring; the ACT ring carries style for the rest.
        EARLY_SP = 2

        @block.sync
        def _(sync):
            for j, (c0, c1) in enumerate(PIECES[:EARLY_SP]):
                sync.dma_start(out=xt[:, c0:c1], in_=x[:, c0:c1]).then_inc(
                    s_x, 16
                )
                sync.dma_start(out=st[:, c0:c1], in_=s[:, c0:c1]).then_inc(
                    s_x, 16
                )
            for (c0, c1) in PIECES[EARLY_SP:]:
                sync.dma_start(out=xt[:, c0:c1], in_=x[:, c0:c1]).then_inc(
                    s_x, 16
                )

        @block.scalar
        def _(scalar):
            for (c0, c1) in PIECES[EARLY_SP:]:
                scalar.dma_start(out=st[:, c0:c1], in_=s[:, c0:c1]).then_inc(
                    s_sv, 16
                )
            for j, (c0, c1) in enumerate(PIECES[:-1]):
                nch = (c1 - c0) // CHUNK
                scalar.wait_ge(s_d, j + 1)
                nc.scalar.activation(
                    out=sq[:, 0:nch],
                    in_=cs[:, c0 // CHUNK : c1 // CHUNK],
                    func=mybir.ActivationFunctionType.Square,
                    accum_out=partials[:, j : j + 1],
                ).then_inc(s_cs, 1)
            scalar.wait_ge(s_cs, N_PIECES)
            scalar.drain()
            scalar.dma_start(out=o, in_=partials[:]).then_inc(s_out, 16)

        @block.gpsimd
        def _(gpsimd):
            for gi, j in enumerate(G_IDX):
                c0, c1 = PIECES[j]
                w = c1 - c0
                gpsimd.wait_ge(s_x, 16 * (j + 1))
                gpsimd.wait_ge(s_sv, 16 * (j + 1))
                nc.gpsimd.tensor_sub(
                    dt_g[:, gi, 0:w], xt[:, c0:c1], st[:, c0:c1]
                ).then_inc(s_g, 1)

        @block.vector
        def _(vector):
            n_g_done = 0
            for j, (c0, c1) in enumerate(PIECES):
                w = c1 - c0
                if OWNER[j] == "G":
                    n_g_done += 1
                    vector.wait_ge(s_g, n_g_done)
                    gi = n_g_done - 1
                    d2 = dt_g[:, gi, 0:w]
                else:
                    if j < EARLY_SP:
                        vector.wait_ge(s_x, 16 * 2 * (j + 1))
                    else:
                        vector.wait_ge(s_x, 16 * (j + EARLY_SP + 1))
                        vector.wait_ge(s_sv, 16 * (j - EARLY_SP + 1))
                    nc.vector.tensor_sub(
                        dt_[:, 0:w], xt[:, c0:c1], st[:, c0:c1]
                    )
                    vector.drain()
                    d2 = dt_[:, 0:w]
                d3 = seg(d2, CHUNK)
                nc.vector.tensor_add(
                    seg(ft[:, 0 : w // 2], 50),
                    d3[:, :, 0:50],
                    d3[:, :, 50:100],
                )
                vector.drain()
                nc.vector.tensor_reduce(
                    out=cs[:, c0 // CHUNK : c1 // CHUNK],
                    in_=seg(ft[:, 0 : w // 2], 50),
                    axis=mybir.AxisListType.X,
                    op=mybir.AluOpType.add,
                ).then_inc(s_d, 1)
            # Final sliver: square+reduce on the DVE.
            last = N_PIECES - 1
            nlast = (PIECES[last][1] - PIECES[last][0]) // CHUNK
            c0l = PIECES[last][0] // CHUNK
            vector.drain()
            nc.vector.tensor_mul(
                sqv[:, 0:nlast],
                cs[:, c0l : c0l + nlast],
                cs[:, c0l : c0l + nlast],
            )
            vector.drain()
            nc.vector.tensor_reduce(
                out=partials[:, last : last + 1],
                in_=sqv[:, 0:nlast],
                axis=mybir.AxisListType.X,
                op=mybir.AluOpType.add,
            ).then_inc(s_cs, 1)

    return nc


def _get_nc():
    global _CACHED_NC
    if _CACHED_NC is None:
        _CACHED_NC = _build_nc()
    return _CACHED_NC


def _prep(arr):
    # [8192, 2500] fp32 -> per-core [128, 20000] bf16, partition-major.
    a = np.asarray(arr, dtype=np.float32).reshape(N_ROWS, K)
    a = a.astype(ml_dtypes.bfloat16)
    a = a.reshape(N_CORES, N_TILES, P, K).transpose(0, 2, 1, 3)
    return np.ascontiguousarray(a).reshape(N_CORES, P, W)


def run_sharded(input, style, **run_kwargs):
    nc = _get_nc()
    xi = _prep(input)
    xs = _prep(style)
    in_maps = [{"input": xi[i], "style": xs[i]} for i in range(N_CORES)]
    res = run_bass_kernel_spmd(nc, in_maps, list(range(N_CORES)), **run_kwargs)
    total = np.float64(0.0)
    for r in res.results:
        total += r["out"].astype(np.float64).sum()
    return np.array(total * SCALE2, dtype=np.float32), res


def kernel(input, style):
    loss, _ = run_sharded(input, style)
    return loss


# revision 20
# speedup vs baseline: 1.0872x; 1.0872x over previous
# Trainium2 Bass kernel for nn_CustomStyleLoss (segment-mean + MSE reduction).
#
# loss = sum_rows mean_chunks( (mean_chunk(input) - mean_chunk(style))^2 )
# with rows = 16*512 = 8192, each row = 50*50 = 2500 elems = 25 chunks of 100.
#
# Data-parallel over rows: core i gets rows [i*1024, (i+1)*1024).
#
# Strategy (memory regime; measured on HW at each step):
# - The 2e-2 tolerance admits bf16 (measured end-to-end rel err 3.6e-5),
#   so the host casts both tensors to bf16, halving HBM traffic to
#   10.24MB/core, and pre-arranges each core shard as [128 partitions,
#   8*2500] (row r=t*128+p -> partition p, column t*2500+k) so every DMA
#   slice is per-partition contiguous at any width.
# - x pieces stream on the SP HWDGE ring, style pieces on the ACT ring
#   (~425-435 GB/s aggregate measured, i.e. ~1.18ns per column of both
#   tensors). Everything fits in SBUF at once (95KB/partition), so DMA
#   free-runs with no slot-reuse backpressure. Piece widths ramp small->
#   large->small: the first compute starts at ~10us instead of ~18.5,
#   and the lands pace the DVE with only ~3us of early idle (each piece
#   boundary also costs ~0.5-1us of completion-receipt lag, so more/
#   smaller pieces are not free - 9 pieces measured best).
# - Compute per piece on the DVE at measured-best perf modes: tensor_sub
#   bf16 (2x_1P, 0.52ns/col), the aligned fold d[...,0:50]+d[...,50:100]
#   (2x even on strided 3D views, 0.26ns/col), tensor_reduce [*,50] (1x
#   uop cap, 0.52ns/elem) with fp32 out - ~1.30ns/col total, formula-
#   exact ((N+151)cyc @0.96GHz per op).
# - Rejected on measurement: fp32/bf16 tensor_tensor_scan (2.11ns/elem
#   regardless of dtype); SWDGE accum_op subtract-in-DMA (CCE RMW runs
#   ~193 GB/s solo, ~40 GB/s while HWDGE is busy, and costs 2x port
#   budget); GpSimd tensor_sub offload (its TT runs 1.92ns/elem but
#   concurrent GpSimd compute slows DVE ops 3.4-6.5x - SBUF access
#   interference - a large net loss).
# - Squares+row-sums run on the ACT engine (Square with accum_out) per
#   piece except the final 4-chunk sliver (DVE, skips one cross-engine
#   hop on the tail). Final scale is applied on the host.

import sys

if "/opt/trn_rl_repo" not in sys.path:
    sys.path.insert(0, "/opt/trn_rl_repo")

import ml_dtypes
import numpy as np

import concourse.bass as bass
from concourse import mybir
from concourse.bass_utils import run_bass_kernel_spmd

N_CORES = 8
N_ROWS = 8192
K = 2500
CHUNK = 100
P = 128
ROWS_PER_CORE = N_ROWS // N_CORES    # 1024
N_TILES = ROWS_PER_CORE // P         # 8
W = N_TILES * K                      # 20000 columns in device layout
N_CHUNKS = W // CHUNK                # 200 chunk sums per partition
SCALE2 = 1.0 / (CHUNK * CHUNK * (K // CHUNK))

# Piece widths (columns, chunk-aligned) and owner of the subtract:
# 'V' = DVE does sub+fold+reduce; 'G' = GpSimd does sub, DVE fold+reduce.
# NOTE: 'G' is now unused - measured GpSimd TT runs concurrently with DVE
# ops slow the DVE 3.4-6.5x (SBUF access interference), a net loss.
# Ramp chosen so each piece's full landing precedes the DVE becoming
# ready for it (no mid-kernel DVE idle), given DVE ~1.30ns/col + 0.77us
# fixed per piece vs DMA ~1.18ns/col.
PIECE_W = [400, 1200, 2400, 3200, 3600, 3600, 3600, 1600, 400]
OWNER =   ["V", "V",  "V",  "V",  "V",  "V",  "V",  "V",  "V"]
assert sum(PIECE_W) == W and all(w % CHUNK == 0 for w in PIECE_W)
N_PIECES = len(PIECE_W)
PIECES = []
_c = 0
for _w in PIECE_W:
    PIECES.append((_c, _c + _w))
    _c += _w
G_IDX = [j for j, o in enumerate(OWNER) if o == "G"]
MAX_GW = max((PIECE_W[j] for j in G_IDX), default=CHUNK)

_CACHED_NC = None


def _build_nc():
    nc = bass.Bass(
        "TRN2", target_bir_lowering=False, debug=False, num_devices=N_CORES
    )
    x = nc.dram_tensor(
        "input", [P, W], mybir.dt.bfloat16, kind="ExternalInput"
    ).ap()
    s = nc.dram_tensor(
        "style", [P, W], mybir.dt.bfloat16, kind="ExternalInput"
    ).ap()
    o = nc.dram_tensor(
        "out", [P, N_PIECES], mybir.dt.float32, kind="ExternalOutput"
    ).ap()

    from contextlib import ExitStack

    with ExitStack() as ctx:
        xt = ctx.enter_context(nc.sbuf_tensor("xt", [P, W], mybir.dt.bfloat16))
        st = ctx.enter_context(nc.sbuf_tensor("st", [P, W], mybir.dt.bfloat16))
        dt_ = ctx.enter_context(
            nc.sbuf_tensor("dt", [P, max(PIECE_W)], mybir.dt.bfloat16)
        )
        ft = ctx.enter_context(
            nc.sbuf_tensor("ft", [P, max(PIECE_W) // 2], mybir.dt.bfloat16)
        )
        # GpSimd-owned pieces get their own sub buffer, slotted per piece
        # so the DVE never races a reuse.
        dt_g = ctx.enter_context(
            nc.sbuf_tensor(
                "dt_g", [P, max(len(G_IDX), 1), MAX_GW], mybir.dt.bfloat16
            )
        )
        cs = ctx.enter_context(
            nc.sbuf_tensor("cs", [P, N_CHUNKS], mybir.dt.float32)
        )
        max_nch = max(PIECE_W) // CHUNK
        sqv = ctx.enter_context(
            nc.sbuf_tensor("sqv", [P, max_nch], mybir.dt.float32)
        )
        sq = ctx.enter_context(
            nc.sbuf_tensor("sq", [P, max_nch], mybir.dt.float32)
        )
        partials = ctx.enter_context(
            nc.sbuf_tensor("partials", [P, N_PIECES], mybir.dt.float32)
        )
        s_x = ctx.enter_context(nc.semaphore("s_x"))
        s_sv = ctx.enter_context(nc.semaphore("s_sv"))
        s_g = ctx.enter_context(nc.semaphore("s_g"))
        s_d = ctx.enter_context(nc.semaphore("s_d"))
        s_cs = ctx.enter_context(nc.semaphore("s_cs"))
        s_out = ctx.enter_context(nc.semaphore("s_out"))
        block = ctx.enter_context(nc.Block(no_gpsimd_drain=True))

        def seg(ap2d, k):  # [P, n*k] -> [P, n, k]
            return ap2d.rearrange("p (c k) -> p c k", k=k)

        @block.sync
        def _(sync):
            for (c0, c1) in PIECES:
                sync.dma_start(out=xt[:, c0:c1], in_=x[:, c0:c1]).then_inc(
                    s_x, 16
                )

        @block.scalar
        def _(scalar):
            for (c0, c1) in PIECES:
                scalar.dma_start(out=st[:, c0:c1], in_=s[:, c0:c1]).then_inc(
                    s_sv, 16
                )
            for j, (c0, c1) in enumerate(PIECES[:-1]):
                nch = (c1 - c0) // CHUNK
                scalar.wait_ge(s_d, j + 1)
                nc.scalar.activation(
                    out=sq[:, 0:nch],
                    in_=cs[:, c0 // CHUNK : c1 // CHUNK],
                    func=mybir.ActivationFunctionType.Square,
                    accum_out=partials[:, j : j + 1],
                ).then_inc(s_cs, 1)
            scalar.wait_ge(s_cs, N_PIECES)
            scalar.drain()
            scalar.dma_start(out=o, in_=partials[:]).then_inc(s_out, 16)

        @block.gpsimd
        def _(gpsimd):
            for gi, j in enumerate(G_IDX):
                c0, c1 = PIECES[j]
                w = c1 - c0
                gpsimd.wait_ge(s_x, 16 * (j + 1))
                gpsimd.wait_ge(s_sv, 16 * (j + 1))
                nc.gpsimd.tensor_sub(
                    dt_g[:, gi, 0:w], xt[:, c0:c1], st[:, c0:c1]
                ).then_inc(s_g, 1)

        @block.vector
        def _(vector):
            n_g_done = 0
            for j, (c0, c1) in enumerate(PIECES):
                w = c1 - c0
                if OWNER[j] == "G":
                    n_g_done += 1
                    vector.wait_ge(s_g, n_g_done)
                    gi = n_g_done - 1
                    d2 = dt_g[:, gi, 0:w]
                else:
                    vector.wait_ge(s_x, 16 * (j + 1))
                    vector.wait_ge(s_sv, 16 * (j + 1))
                    nc.vector.tensor_sub(
                        dt_[:, 0:w], xt[:, c0:c1], st[:, c0:c1]
                    )
                    vector.drain()
                    d2 = dt_[:, 0:w]
                d3 = seg(d2, CHUNK)
                nc.vector.tensor_add(
                    seg(ft[:, 0 : w // 2], 50),
                    d3[:, :, 0:50],
                    d3[:, :, 50:100],
                )
                vector.drain()
                nc.vector.tensor_reduce(
                    out=cs[:, c0 // CHUNK : c1 // CHUNK],
                    in_=seg(ft[:, 0 : w // 2], 50),
                    axis=mybir.AxisListType.X,
                    op=mybir.AluOpType.add,
                ).then_inc(s_d, 1)
            # Final sliver: square+reduce on the DVE.
            last = N_PIECES - 1
            nlast = (PIECES[last][1] - PIECES[last][0]) // CHUNK
            c0l = PIECES[last][0] // CHUNK
            vector.drain()
            nc.vector.tensor_mul(
                sqv[:, 0:nlast],
                cs[:, c0l : c0l + nlast],
                cs[:, c0l : c0l + nlast],
            )
            vector.drain()
            nc.vector.tensor_reduce(
                out=partials[:, last : last + 1],
                in_=sqv[:, 0:nlast],
                axis=mybir.AxisListType.X,
                op=mybir.AluOpType.add,
            ).then_inc(s_cs, 1)

    return nc


def _get_nc():
    global _CACHED_NC
    if _CACHED_NC is None:
        _CACHED_NC = _build_nc()
    return _CACHED_NC


def _prep(arr):
    # [8192, 2500] fp32 -> per-core [128, 20000] bf16, partition-major.
    a = np.asarray(arr, dtype=np.float32).reshape(N_ROWS, K)
    a = a.astype(ml_dtypes.bfloat16)
    a = a.reshape(N_CORES, N_TILES, P, K).transpose(0, 2, 1, 3)
    return np.ascontiguousarray(a).reshape(N_CORES, P, W)


def run_sharded(input, style, **run_kwargs):
    nc = _get_nc()
    xi = _prep(input)
    xs = _prep(style)
    in_maps = [{"input": xi[i], "style": xs[i]} for i in range(N_CORES)]
    res = run_bass_kernel_spmd(nc, in_maps, list(range(N_CORES)), **run_kwargs)
    total = np.float64(0.0)
    for r in res.results:
        total += r["out"].astype(np.float64).sum()
    return np.array(total * SCALE2, dtype=np.float32), res


def kernel(input, style):
    loss, _ = run_sharded(input, style)
    return loss


# revision 21
# speedup vs baseline: 1.1181x; 1.0285x over previous
# Trainium2 Bass kernel for nn_CustomStyleLoss (segment-mean + MSE reduction).
#
# loss = sum_rows mean_chunks( (mean_chunk(input) - mean_chunk(style))^2 )
# with rows = 16*512 = 8192, each row = 50*50 = 2500 elems = 25 chunks of 100.
#
# Data-parallel over rows: core i gets rows [i*1024, (i+1)*1024).
#
# Strategy (memory regime; measured on HW at each step):
# - The 2e-2 tolerance admits bf16 (measured end-to-end rel err 3.6e-5),
#   so the host casts both tensors to bf16, halving HBM traffic to
#   10.24MB/core, and pre-arranges each core shard as [128 partitions,
#   8*2500] (row r=t*128+p -> partition p, column t*2500+k) so every DMA
#   slice is per-partition contiguous at any width.
# - x pieces stream on the SP HWDGE ring, style pieces on the ACT ring
#   (~425-435 GB/s aggregate measured, i.e. ~1.18ns per column of both
#   tensors). Everything fits in SBUF at once (95KB/partition), so DMA
#   free-runs with no slot-reuse backpressure. Piece widths ramp small->
#   large->small: the first compute starts at ~10us instead of ~18.5,
#   and the lands pace the DVE with only ~3us of early idle (each piece
#   boundary also costs ~0.5-1us of completion-receipt lag, so more/
#   smaller pieces are not free - 9 pieces measured best).
# - Compute per piece on the DVE at measured-best perf modes: tensor_sub
#   bf16 (2x_1P, 0.52ns/col), the aligned fold d[...,0:50]+d[...,50:100]
#   (2x even on strided 3D views, 0.26ns/col), tensor_reduce [*,50] (1x
#   uop cap, 0.52ns/elem) with fp32 out - ~1.30ns/col total, formula-
#   exact ((N+151)cyc @0.96GHz per op).
# - Rejected on measurement: fp32/bf16 tensor_tensor_scan (2.11ns/elem
#   regardless of dtype); SWDGE accum_op subtract-in-DMA (CCE RMW runs
#   ~193 GB/s solo, ~40 GB/s while HWDGE is busy, and costs 2x port
#   budget); GpSimd tensor_sub offload (its TT runs 1.92ns/elem but
#   concurrent GpSimd compute slows DVE ops 3.4-6.5x - SBUF access
#   interference - a large net loss).
# - Squares+row-sums run on the ACT engine (Square with accum_out) per
#   piece except the final 4-chunk sliver (DVE, skips one cross-engine
#   hop on the tail). Final scale is applied on the host.

import sys

if "/opt/trn_rl_repo" not in sys.path:
    sys.path.insert(0, "/opt/trn_rl_repo")

import ml_dtypes
import numpy as np

import concourse.bass as bass
from concourse import mybir
from concourse.bass_utils import run_bass_kernel_spmd

N_CORES = 8
N_ROWS = 8192
K = 2500
CHUNK = 100
P = 128
ROWS_PER_CORE = N_ROWS // N_CORES    # 1024
N_TILES = ROWS_PER_CORE // P         # 8
W = N_TILES * K                      # 20000 columns in device layout
N_CHUNKS = W // CHUNK                # 200 chunk sums per partition
SCALE2 = 1.0 / (CHUNK * CHUNK * (K // CHUNK))

# Piece widths (columns, chunk-aligned) and owner of the subtract:
# 'V' = DVE does sub+fold+reduce; 'G' = GpSimd does sub, DVE fold+reduce.
# NOTE: 'G' is now unused - measured GpSimd TT runs concurrently with DVE
# ops slow the DVE 3.4-6.5x (SBUF access interference), a net loss.
# Ramp chosen so each piece's full landing precedes the DVE becoming
# ready for it (no mid-kernel DVE idle), given DVE ~1.30ns/col + 0.77us
# fixed per piece vs DMA ~1.18ns/col.
PIECE_W = [400, 1200, 2400, 3200, 3600, 3600, 3600, 2000]
OWNER =   ["V", "V",  "V",  "V",  "V",  "V",  "V",  "V"]
assert sum(PIECE_W) == W and all(w % CHUNK == 0 for w in PIECE_W)
N_PIECES = len(PIECE_W)
PIECES = []
_c = 0
for _w in PIECE_W:
    PIECES.append((_c, _c + _w))
    _c += _w
G_IDX = [j for j, o in enumerate(OWNER) if o == "G"]
MAX_GW = max((PIECE_W[j] for j in G_IDX), default=CHUNK)

_CACHED_NC = None


def _build_nc():
    nc = bass.Bass(
        "TRN2", target_bir_lowering=False, debug=False, num_devices=N_CORES
    )
    x = nc.dram_tensor(
        "input", [P, W], mybir.dt.bfloat16, kind="ExternalInput"
    ).ap()
    s = nc.dram_tensor(
        "style", [P, W], mybir.dt.bfloat16, kind="ExternalInput"
    ).ap()
    o = nc.dram_tensor(
        "out", [P, N_PIECES], mybir.dt.float32, kind="ExternalOutput"
    ).ap()

    from contextlib import ExitStack

    with ExitStack() as ctx:
        xt = ctx.enter_context(nc.sbuf_tensor("xt", [P, W], mybir.dt.bfloat16))
        st = ctx.enter_context(nc.sbuf_tensor("st", [P, W], mybir.dt.bfloat16))
        dt_ = ctx.enter_context(
            nc.sbuf_tensor("dt", [P, max(PIECE_W)], mybir.dt.bfloat16)
        )
        ft = ctx.enter_context(
            nc.sbuf_tensor("ft", [P, max(PIECE_W) // 2], mybir.dt.bfloat16)
        )
        # GpSimd-owned pieces get their own sub buffer, slotted per piece
        # so the DVE never races a reuse.
        dt_g = ctx.enter_context(
            nc.sbuf_tensor(
                "dt_g", [P, max(len(G_IDX), 1), MAX_GW], mybir.dt.bfloat16
            )
        )
        cs = ctx.enter_context(
            nc.sbuf_tensor("cs", [P, N_CHUNKS], mybir.dt.float32)
        )
        max_nch = max(PIECE_W) // CHUNK
        sqv = ctx.enter_context(
            nc.sbuf_tensor("sqv", [P, max_nch], mybir.dt.float32)
        )
        sq = ctx.enter_context(
            nc.sbuf_tensor("sq", [P, max_nch], mybir.dt.float32)
        )
        partials = ctx.enter_context(
            nc.sbuf_tensor("partials", [P, N_PIECES], mybir.dt.float32)
        )
        s_x = ctx.enter_context(nc.semaphore("s_x"))
        s_sv = ctx.enter_context(nc.semaphore("s_sv"))
        s_g = ctx.enter_context(nc.semaphore("s_g"))
        s_d = ctx.enter_context(nc.semaphore("s_d"))
        s_cs = ctx.enter_context(nc.semaphore("s_cs"))
        s_out = ctx.enter_context(nc.semaphore("s_out"))
        block = ctx.enter_context(nc.Block(no_gpsimd_drain=True))

        def seg(ap2d, k):  # [P, n*k] -> [P, n, k]
            return ap2d.rearrange("p (c k) -> p c k", k=k)

        @block.sync
        def _(sync):
            for (c0, c1) in PIECES:
                sync.dma_start(out=xt[:, c0:c1], in_=x[:, c0:c1]).then_inc(
                    s_x, 16
                )

        @block.scalar
        def _(scalar):
            for (c0, c1) in PIECES:
                scalar.dma_start(out=st[:, c0:c1], in_=s[:, c0:c1]).then_inc(
                    s_sv, 16
                )
            for j, (c0, c1) in enumerate(PIECES[:-1]):
                nch = (c1 - c0) // CHUNK
                scalar.wait_ge(s_d, j + 1)
                nc.scalar.activation(
                    out=sq[:, 0:nch],
                    in_=cs[:, c0 // CHUNK : c1 // CHUNK],
                    func=mybir.ActivationFunctionType.Square,
                    accum_out=partials[:, j : j + 1],
                ).then_inc(s_cs, 1)
            scalar.wait_ge(s_cs, N_PIECES)
            scalar.drain()
            scalar.dma_start(out=o, in_=partials[:]).then_inc(s_out, 16)

        @block.gpsimd
        def _(gpsimd):
            for gi, j in enumerate(G_IDX):
                c0, c1 = PIECES[j]
                w = c1 - c0
                gpsimd.wait_ge(s_x, 16 * (j + 1))
                gpsimd.wait_ge(s_sv, 16 * (j + 1))
                nc.gpsimd.tensor_sub(
                    dt_g[:, gi, 0:w], xt[:, c0:c1], st[:, c0:c1]
                ).then_inc(s_g, 1)

        @block.vector
        def _(vector):
            n_g_done = 0
            for j, (c0, c1) in enumerate(PIECES):
                w = c1 - c0
                if OWNER[j] == "G":
                    n_g_done += 1
                    vector.wait_ge(s_g, n_g_done)
                    gi = n_g_done - 1
                    d2 = dt_g[:, gi, 0:w]
                else:
                    vector.wait_ge(s_x, 16 * (j + 1))
                    vector.wait_ge(s_sv, 16 * (j + 1))
                    nc.vector.tensor_sub(
                        dt_[:, 0:w], xt[:, c0:c1], st[:, c0:c1]
                    )
                    vector.drain()
                    d2 = dt_[:, 0:w]
                d3 = seg(d2, CHUNK)
                nc.vector.tensor_add(
                    seg(ft[:, 0 : w // 2], 50),
                    d3[:, :, 0:50],
                    d3[:, :, 50:100],
                )
                vector.drain()
                nc.vector.tensor_reduce(
                    out=cs[:, c0 // CHUNK : c1 // CHUNK],
                    in_=seg(ft[:, 0 : w // 2], 50),
                    axis=mybir.AxisListType.X,
                    op=mybir.AluOpType.add,
                ).then_inc(s_d, 1)
            # Final sliver: square+reduce on the DVE.
            last = N_PIECES - 1
            nlast = (PIECES[last][1] - PIECES[last][0]) // CHUNK
            c0l = PIECES[last][0] // CHUNK
            vector.drain()
            nc.vector.tensor_mul(
                sqv[:, 0:nlast],
                cs[:, c0l : c0l + nlast],
                cs[:, c0l : c0l + nlast],
            )
            vector.drain()
            nc.vector.tensor_reduce(
                out=partials[:, last : last + 1],
                in_=sqv[:, 0:nlast],
                axis=mybir.AxisListType.X,
                op=mybir.AluOpType.add,
            ).then_inc(s_cs, 1)

    return nc


def _get_nc():
    global _CACHED_NC
    if _CACHED_NC is None:
        _CACHED_NC = _build_nc()
    return _CACHED_NC


def _prep(arr):
    # [8192, 2500] fp32 -> per-core [128, 20000] bf16, partition-major.
    a = np.asarray(arr, dtype=np.float32).reshape(N_ROWS, K)
    a = a.astype(ml_dtypes.bfloat16)
    a = a.reshape(N_CORES, N_TILES, P, K).transpose(0, 2, 1, 3)
    return np.ascontiguousarray(a).reshape(N_CORES, P, W)


def run_sharded(input, style, **run_kwargs):
    nc = _get_nc()
    xi = _prep(input)
    xs = _prep(style)
    in_maps = [{"input": xi[i], "style": xs[i]} for i in range(N_CORES)]
    res = run_bass_kernel_spmd(nc, in_maps, list(range(N_CORES)), **run_kwargs)
    total = np.float64(0.0)
    for r in res.results:
        total += r["out"].astype(np.float64).sum()
    return np.array(total * SCALE2, dtype=np.float32), res


def kernel(input, style):
    loss, _ = run_sharded(input, style)
    return loss
